# revision 7
# baseline (speedup 1.0000x reference)
"""JKNet (5-layer GCN + JumpingKnowledge-max + linear head) on 8 Trainium2 cores.

Strategy (dst-sharded message passing, v6 - batched SWDGE gathers + fp8
one-hot scatter):
  - Nodes are sharded contiguously across 8 cores (12500 per core).
  - The GCN norm dinv_d*dinv_s is factorized: the node-feature tables store
    T[s] = dinv_s * h[s] (applied for free on the table-write ACT copy), and
    dinv_d is applied per dst tile with one DVE multiply against a resident
    row-broadcast tile.  The scatter matrix S[e,d] = (dst_e == d) is then an
    EXACT 0/1 one-hot stored in fp8 (half the DVE build cost; PE runs a
    mixed bf16 x fp8 matmul into fp32 PSUM).
  - Self-loops use a dense path: one contiguous bulk DMA of the core's own
    node-major shard + a resident fp8 identity as S (no per-row gather
    descriptors, no per-chunk S build).
  - Edges are partitioned by (dst 128-row tile, 32768-row source window of
    the bf16 replica); per-bucket counts are equalized to the max over the
    8 cores so all cores share one program.  The first n_win-1 windows are
    chunk-aligned per bucket; the tiny last window is packed across the
    tiles of each group (chunks cross tile boundaries; lanes of other tiles
    are masked with dst = -1).
  - Per group of 12 dst tiles x window: ONE nc.gpsimd.dma_gather call
    (int16 local indices into the 32768-row window view of the replica)
    amortizes the ~1us SWDGE fixed overhead and runs the Q7
    descriptor-generation loop once per ~4600 rows instead of per 128.
  - Per tile: q^T = psq * dinv_d (DVE, row-broadcast), W matmul (bf16),
    fused BN+ReLU on ACT (bf16), JumpingKnowledge running max (DVE), PE
    transpose, ACT copy scaled by dinv_d -> T, DMA to the AllGather input.
  - The per-layer AllGather of the bf16 replica is split into 4 pieces
    issued progressively so all but a small tail overlaps with compute.
  - Head: logits = hmax^T.T @ lin_w per tile (bf16), + bias, log_softmax
    (fp32), DMA the core's [12500, 40] shard out.
"""

import math
import os

import numpy as np

import concourse.bass as bass
import concourse.mybir as mybir
import concourse.tile as tile
from concourse import bacc
from concourse.bass_utils import run_bass_kernel_spmd
from concourse.masks import make_identity

P = 128          # partitions / feature dim / edge-chunk size
NCORES = 8
BN_EPS = 1e-5
WINDOW = 32768   # int16-indexable rows per gather source window
G_TILES = 8      # dst tiles per gather group


# ---------------------------------------------------------------- host prep
def preprocess_edges(edge_index, n_nodes, ncores=NCORES):
    """Partition regular edges by (dst tile, src window); self-loops excluded
    (they use the dense path).

    Returns (per_core, layout):
      per_core: 'eidx' [128, slots//16] int16, 'edst' [P, n_segs] f32 (dst
                lane or -1 per segment column)
      layout:   shared program structure.
    """
    row = np.asarray(edge_index[0], dtype=np.int64)   # dst
    col = np.asarray(edge_index[1], dtype=np.int64)   # src
    deg = np.bincount(row, minlength=n_nodes).astype(np.float64) + 1.0
    dinv = (1.0 / np.sqrt(deg)).astype(np.float32)

    sh = n_nodes // ncores
    t_tiles = math.ceil(sh / P)
    n_win = math.ceil(n_nodes / WINDOW)

    # AllGather piece boundaries (local rows, tile-aligned)
    piece_tiles = sorted({0, min(32, t_tiles), min(64, t_tiles),
                          max(t_tiles - 7, 0), t_tiles})
    bounds = np.array(sorted({min(t * P, sh) for t in piece_tiles}),
                      dtype=np.int64)
    plen = bounds[1:] - bounds[:-1]

    # replica position of every node (piece-major split-AllGather layout)
    n_arr = np.arange(n_nodes, dtype=np.int64)
    c_ = n_arr // sh
    r_ = n_arr % sh
    pi = np.searchsorted(bounds[1:], r_, side="right")
    posarr = ncores * bounds[pi] + c_ * plen[pi] + (r_ - bounds[pi])

    # per-core edge lists sorted by (tile, win, src)
    counts = np.zeros((ncores, t_tiles, n_win), np.int64)
    percore_sorted = []
    for c in range(ncores):
        m = (row // sh) == c
        d = row[m] - c * sh
        s = col[m]
        spos = posarr[s]
        tid = d // P
        win = spos // WINDOW
        order = np.lexsort((spos, win, tid))
        tid_s, d_s, spos_s = tid[order], d[order], spos[order]
        counts[c] = np.bincount(tid_s * n_win + win[order],
                                minlength=t_tiles * n_win).reshape(t_tiles,
                                                                   n_win)
        percore_sorted.append(((d_s % P).astype(np.float32), spos_s))

    ncnt = counts.max(axis=0)                   # equalized bucket sizes

    groups = [list(range(g * G_TILES, min((g + 1) * G_TILES, t_tiles)))
              for g in range(math.ceil(t_tiles / G_TILES))]

    # slot layout + segment structure (shared across cores)
    # segments: (tile, w, jj_in_call, ci) ; chunk lanes map to slots
    #   [chunk_lo, chunk_lo+128); the segment covers bucket slots
    #   [b_lo, b_lo + ncnt) clipped to the chunk; per-core valid edges are
    #   the first counts[c] slots of the bucket, the rest masked dst=-1.
    cur = 0
    n_segs = 0
    call_info = []           # per group: list of (w, slot_off, n_idx)
    tile_segs = [[] for _ in range(t_tiles)]
    seg_fill = []            # (ci, t, w, chunk_lo, b_lo)
    bucket_off = np.zeros((t_tiles, n_win), np.int64)
    for grp in groups:
        gi = []
        for w in range(n_win):
            call_off = cur
            if w < n_win - 1:
                # aligned: each bucket rounded to whole chunks
                for t in grp:
                    bucket_off[t, w] = cur
                    k = -(-int(ncnt[t, w]) // P)
                    for j in range(k):
                        chunk_lo = cur + j * P
                        jj = (chunk_lo - call_off) // P
                        tile_segs[t].append((w, jj, n_segs))
                        seg_fill.append((n_segs, t, w, chunk_lo, cur))
                        n_segs += 1
                    cur += k * P
            else:
                # packed: buckets concatenated, chunks cross tiles
                b_lo = {}
                for t in grp:
                    bucket_off[t, w] = cur
                    b_lo[t] = cur
                    cur += int(ncnt[t, w])
                cur = -(-cur // P) * P          # call tail pad
                nch = (cur - call_off) // P
                for j in range(nch):
                    chunk_lo = call_off + j * P
                    chunk_hi = chunk_lo + P
                    for t in grp:
                        lo = b_lo[t]
                        hi = lo + int(ncnt[t, w])
                        if lo < chunk_hi and hi > chunk_lo:
                            jj = j
                            tile_segs[t].append((w, jj, n_segs))
                            seg_fill.append((n_segs, t, w, chunk_lo, lo))
                            n_segs += 1
            gi.append((w, call_off, cur - call_off))
        call_info.append(gi)
    total_slots = cur

    per_core = []
    for c in range(ncores):
        din, spos = percore_sorted[c]
        cstart = np.concatenate([[0], np.cumsum(counts[c].reshape(-1))])
        idxf = np.zeros(total_slots, np.int16)
        for t in range(t_tiles):
            for w in range(n_win):
                n = int(counts[c][t, w])
                o = int(bucket_off[t, w])
                cap = int(ncnt[t, w])
                if w < n_win - 1:
                    cap = -(-cap // P) * P
                if n:
                    a = int(cstart[t * n_win + w])
                    idxf[o:o + n] = spos[a:a + n] - w * WINDOW
                    if n < cap:
                        idxf[o + n:o + cap] = idxf[o + n - 1]
        dstm = np.full((n_segs, P), -1.0, np.float32)
        for ci, t, w, chunk_lo, b_lo in seg_fill:
            n = int(counts[c][t, w])
            a = int(cstart[t * n_win + w])
            lo = max(chunk_lo, b_lo)
            hi = min(chunk_lo + P, b_lo + n)
            if hi > lo:
                dstm[ci, lo - chunk_lo:hi - chunk_lo] = \
                    din[a + (lo - b_lo):a + (hi - b_lo)]
        per_core.append({
            "eidx": np.ascontiguousarray(
                np.tile(idxf.reshape(-1, 16).T, (8, 1))),
            "edst": np.ascontiguousarray(dstm.T),
        })

    layout = dict(
        groups=groups, call_info=call_info, tile_segs=tile_segs,
        total_slots=total_slots, n_segs=n_segs,
        bounds=bounds, posarr=posarr, n_win=n_win, t_tiles=t_tiles,
        dinv=dinv,
    )
    return per_core, layout


# ---------------------------------------------------------------- program
def build_program(n_nodes, n_layers, n_cls, layout, ncores=NCORES):
    f32 = mybir.dt.float32
    bf16 = mybir.dt.bfloat16
    fp8 = mybir.dt.float8e4
    sh = n_nodes // ncores
    t_tiles = layout["t_tiles"]
    n_win = layout["n_win"]
    groups = layout["groups"]
    call_info = layout["call_info"]
    tile_segs = layout["tile_segs"]
    total_slots = layout["total_slots"]
    n_segs = layout["n_segs"]
    bounds = layout["bounds"]
    win_lo = [w * WINDOW for w in range(n_win)]
    win_hi = [min((w + 1) * WINDOW, n_nodes) for w in range(n_win)]
    kmax = [max((ni // P) for gi in call_info for (w2, _, ni) in gi
                if w2 == w) for w in range(n_win)]
    piece_group = [(p, math.ceil(int(bounds[p + 1]) / P / G_TILES) - 1)
                   for p in range(len(bounds) - 1)]

    nc = bacc.Bacc("TRN2", target_bir_lowering=False, debug=False,
                   num_devices=ncores, num_swdge_queues=4)
    xb_t = nc.dram_tensor("xbf", [n_nodes, P], bf16, kind="ExternalInput")
    xs_t = nc.dram_tensor("xself", [sh, P], bf16, kind="ExternalInput")
    idx_t = nc.dram_tensor("eidx", [P, total_slots // 16], mybir.dt.int16,
                           kind="ExternalInput")
    dst_t = nc.dram_tensor("edst", [P, n_segs], f32, kind="ExternalInput")
    dvT_t = nc.dram_tensor("dinvT", [P, t_tiles * P], bf16,
                           kind="ExternalInput")
    dv_t = nc.dram_tensor("edinv", [P, t_tiles], f32, kind="ExternalInput")
    w_t = nc.dram_tensor("conv_w", [n_layers, P, P], f32, kind="ExternalInput")
    cb_t = nc.dram_tensor("conv_b", [n_layers, P], f32, kind="ExternalInput")
    gam_t = nc.dram_tensor("bn_gamma", [n_layers, P], f32, kind="ExternalInput")
    bet_t = nc.dram_tensor("bn_beta", [n_layers, P], f32, kind="ExternalInput")
    mu_t = nc.dram_tensor("bn_mean", [n_layers, P], f32, kind="ExternalInput")
    var_t = nc.dram_tensor("bn_var", [n_layers, P], f32, kind="ExternalInput")
    lw_t = nc.dram_tensor("lin_w", [P, n_cls], f32, kind="ExternalInput")
    lb_t = nc.dram_tensor("lin_b_rep", [P, n_cls], f32, kind="ExternalInput")
    out_t = nc.dram_tensor("out", [sh, n_cls], f32, kind="ExternalOutput")

    hbuf = [nc.dram_tensor(f"hbuf{l}", [n_nodes, P], bf16, addr_space="Shared")
            for l in range(1, n_layers)]
    hbuf = [xb_t] + hbuf
    ag_in = [nc.dram_tensor(f"ag_in{l}", [sh, P], bf16)
             for l in range(n_layers - 1)]
    rgroups = [list(range(ncores))]
    AF = mybir.ActivationFunctionType
    OP = mybir.AluOpType

    # per-group segment list in S-array order + per-tile slot ranges
    gsegs = []               # per group: [(t, w, jj, ci), ...]
    tile_srange = [None] * t_tiles   # (group, start, count)
    for g, grp in enumerate(groups):
        lst = []
        for t in grp:
            tile_srange[t] = (g, len(lst), len(tile_segs[t]))
            lst.extend((t,) + s for s in tile_segs[t])
        gsegs.append(lst)
    segmax = max(len(lst) for lst in gsegs)

    with tile.TileContext(nc) as tc:
        with tc.tile_pool(name="const", bufs=1) as cpool, \
             tc.tile_pool(name="edges", bufs=1) as epool, \
             tc.tile_pool(name="msgs", bufs=2) as mpool, \
             tc.tile_pool(name="selfp", bufs=2) as fpool, \
             tc.tile_pool(name="spool", bufs=2) as spool, \
             tc.tile_pool(name="work", bufs=3) as wpool, \
             tc.tile_pool(name="psum", bufs=2, space="PSUM") as pspool, \
             tc.tile_pool(name="psumq", bufs=4, space="PSUM") as pqpool:

            # -------- resident edge data + constants
            idx_sb = epool.tile([P, total_slots // 16], mybir.dt.int16)
            dst_sb = epool.tile([P, n_segs], f32)
            dvT_sb = epool.tile([P, t_tiles * P], bf16)
            dv_sb = epool.tile([P, t_tiles], f32)
            nc.sync.dma_start(out=idx_sb[:], in_=idx_t[:])
            nc.sync.dma_start(out=dst_sb[:], in_=dst_t[:])
            nc.sync.dma_start(out=dvT_sb[:], in_=dvT_t[:])
            nc.sync.dma_start(out=dv_sb[:], in_=dv_t[:])

            iota_i = cpool.tile([P, P], mybir.dt.int32)
            nc.gpsimd.iota(iota_i[:], pattern=[[1, P]], base=0,
                           channel_multiplier=0)
            iota_b = cpool.tile([P, P], bf16)
            nc.vector.tensor_copy(iota_b[:], iota_i[:])
            ident = cpool.tile([P, P], bf16)
            make_identity(nc, ident[:])
            ident_f8 = cpool.tile([P, P], fp8)
            nc.vector.tensor_copy(ident_f8[:], ident[:])

            w_sb = []
            for l in range(n_layers):
                wf = cpool.tile([P, P], f32, tag=f"wf{l}")
                nc.sync.dma_start(out=wf[:], in_=w_t[l, :, :])
                wl = cpool.tile([P, P], bf16, tag=f"w{l}")
                nc.vector.tensor_copy(wl[:], wf[:])
                w_sb.append(wl)
            lwf = cpool.tile([P, n_cls], f32)
            nc.sync.dma_start(out=lwf[:], in_=lw_t[:])
            lw_sb = cpool.tile([P, n_cls], bf16)
            nc.vector.tensor_copy(lw_sb[:], lwf[:])
            lb_sb = cpool.tile([P, n_cls], f32)
            nc.sync.dma_start(out=lb_sb[:], in_=lb_t[:])

            # -------- BN constants per layer: scale s = gamma * rsqrt(var+eps)
            #          shift = s*(conv_b - mean) + beta      (feature-major [P,1])
            s_sb, sh_sb = [], []
            for l in range(n_layers):
                g_ = cpool.tile([P, 1], f32, tag=f"bng{l}")
                b_ = cpool.tile([P, 1], f32, tag=f"bnb{l}")
                m_ = cpool.tile([P, 1], f32, tag=f"bnm{l}")
                v_ = cpool.tile([P, 1], f32, tag=f"bnv{l}")
                cb_ = cpool.tile([P, 1], f32, tag=f"bnc{l}")
                nc.sync.dma_start(out=g_[:], in_=gam_t[l, :, None])
                nc.sync.dma_start(out=b_[:], in_=bet_t[l, :, None])
                nc.sync.dma_start(out=m_[:], in_=mu_t[l, :, None])
                nc.sync.dma_start(out=v_[:], in_=var_t[l, :, None])
                nc.sync.dma_start(out=cb_[:], in_=cb_t[l, :, None])
                ve = cpool.tile([P, 1], f32, tag=f"bnve{l}")
                nc.vector.tensor_scalar_add(ve[:], v_[:], BN_EPS)
                nc.scalar.sqrt(ve[:], ve[:])
                rv = cpool.tile([P, 1], f32, tag=f"bnrv{l}")
                nc.vector.reciprocal(rv[:], ve[:])
                s_ = cpool.tile([P, 1], f32, tag=f"bns{l}")
                nc.vector.tensor_tensor(out=s_[:], in0=g_[:], in1=rv[:], op=OP.mult)
                d_ = cpool.tile([P, 1], f32, tag=f"bnd{l}")
                nc.vector.tensor_tensor(out=d_[:], in0=cb_[:], in1=m_[:], op=OP.subtract)
                t_ = cpool.tile([P, 1], f32, tag=f"bnt{l}")
                nc.vector.tensor_tensor(out=t_[:], in0=d_[:], in1=s_[:], op=OP.mult)
                nc.vector.tensor_tensor(out=t_[:], in0=t_[:], in1=b_[:], op=OP.add)
                s_sb.append(s_)
                sh_sb.append(t_)

            hmax = epool.tile([P, t_tiles * P], bf16)
            nc.vector.memset(hmax[:], 0.0)

            # -------- layers (software-pipelined by one group: phase A of
            # group g — gathers, self DMAs, S builds — runs while phase B of
            # group g-1 — matmuls + per-tile tail — consumes the previous
            # buffers, so DVE/PE never wait on in-flight gathers)
            n_groups = len(groups)
            for l in range(n_layers):
                table = hbuf[l]
                table_self = xs_t if l == 0 else ag_in[l - 1]
                mw_st = [None, None]
                ms_st = [None, None]
                sa_st = [None, None]
                for ph in range(n_groups + 1):
                    if ph < n_groups:
                        g, grp = ph, groups[ph]
                        pb = ph % 2
                        mw = {}
                        for w, call_off, ni in call_info[g]:
                            if ni == 0:
                                continue
                            mt = mpool.tile([P, kmax[w], P], bf16, tag=f"m{w}")
                            nc.gpsimd.dma_gather(
                                mt[:, :ni // P, :],
                                table[win_lo[w]:win_hi[w], :],
                                idx_sb[:, call_off // 16:(call_off + ni) // 16],
                                ni, ni, P, queue_num=w, single_packet=False)
                            mw[w] = mt
                        mw_st[pb] = mw
                        msl = []
                        for i, t in enumerate(grp):
                            rows = min(P, sh - t * P)
                            ms = fpool.tile([P, P], bf16, tag=f"ms{i}")
                            nc.sync.dma_start(
                                out=ms[:rows, :],
                                in_=table_self[t * P:t * P + rows, :])
                            msl.append(ms)
                        ms_st[pb] = msl
                        sa = spool.tile([P, segmax, P], fp8, tag="sarr")
                        for k, (t, w, jj, ci) in enumerate(gsegs[g]):
                            nc.vector.tensor_scalar(
                                out=sa[:, k, :], in0=iota_b[:],
                                scalar1=dst_sb[:, ci:ci + 1], scalar2=None,
                                op0=OP.is_equal)
                        sa_st[pb] = sa
                    if ph >= 1:
                        g, grp = ph - 1, groups[ph - 1]
                        pb = (ph - 1) % 2
                        mw, msl, sa = mw_st[pb], ms_st[pb], sa_st[pb]
                        for i, t in enumerate(grp):
                            _, sstart, scount = tile_srange[t]
                            rows = min(P, sh - t * P)
                            psq = pqpool.tile([P, P], f32, tag="q", space="PSUM")
                            nc.tensor.matmul(psq[:], lhsT=msl[i][:],
                                             rhs=ident_f8[:],
                                             start=True, stop=(scount == 0))
                            for si in range(scount):
                                t2, w, jj, ci = gsegs[g][sstart + si]
                                nc.tensor.matmul(
                                    psq[:], lhsT=mw[w][:, jj, :],
                                    rhs=sa[:, sstart + si, :],
                                    start=False, stop=(si == scount - 1))
                            q_sb = wpool.tile([P, P], bf16, tag="qT")
                            nc.vector.tensor_tensor(
                                out=q_sb[:], in0=psq[:],
                                in1=dvT_sb[:, t * P:(t + 1) * P], op=OP.mult)
                            phh = pspool.tile([P, P], f32, tag="h", space="PSUM")
                            nc.tensor.matmul(phh[:], lhsT=w_sb[l][:], rhs=q_sb[:],
                                             start=True, stop=True)
                            h_t = wpool.tile([P, P], bf16, tag="hT")
                            nc.scalar.activation(h_t[:], phh[:], AF.Relu,
                                                 bias=sh_sb[l][:, :1],
                                                 scale=s_sb[l][:, :1])
                            nc.vector.tensor_tensor(
                                out=hmax[:, t * P:(t + 1) * P],
                                in0=hmax[:, t * P:(t + 1) * P], in1=h_t[:],
                                op=OP.max)
                            if l < n_layers - 1:
                                pt = pspool.tile([P, P], bf16, tag="t", space="PSUM")
                                nc.tensor.transpose(pt[:], h_t[:], ident[:])
                                hn = wpool.tile([P, P], bf16, tag="hn")
                                nc.scalar.activation(hn[:], pt[:], AF.Copy,
                                                     scale=dv_sb[:, t:t + 1])
                                nc.sync.dma_start(
                                    out=ag_in[l][t * P:t * P + rows, :],
                                    in_=hn[:rows, :])
                            else:
                                po = pspool.tile([P, n_cls], f32, tag="h",
                                                 space="PSUM")
                                nc.tensor.matmul(po[:],
                                                 lhsT=hmax[:, t * P:(t + 1) * P],
                                                 rhs=lw_sb[:], start=True,
                                                 stop=True)
                                z = wpool.tile([P, n_cls], f32, tag="z")
                                nc.vector.tensor_tensor(out=z[:], in0=po[:],
                                                        in1=lb_sb[:], op=OP.add)
                                nm = wpool.tile([P, 1], f32, tag="nm")
                                nc.vector.reduce_max(nm[:], z[:],
                                                     axis=mybir.AxisListType.X,
                                                     negate=True)
                                ez = wpool.tile([P, n_cls], f32, tag="ez")
                                nc.scalar.activation(ez[:], z[:], AF.Exp,
                                                     bias=nm[:, :1], scale=1.0)
                                ss2 = wpool.tile([P, 1], f32, tag="ss2")
                                nc.vector.reduce_sum(ss2[:], ez[:],
                                                     axis=mybir.AxisListType.X)
                                ls = wpool.tile([P, 1], f32, tag="ls")
                                nc.scalar.activation(ls[:], ss2[:], AF.Ln)
                                oz = wpool.tile([P, n_cls], f32, tag="oz")
                                nc.vector.tensor_scalar(out=oz[:], in0=z[:],
                                                        scalar1=nm[:, :1],
                                                        scalar2=ls[:, :1],
                                                        op0=OP.add,
                                                        op1=OP.subtract)
                                nc.sync.dma_start(
                                    out=out_t[t * P:t * P + rows, :],
                                    in_=oz[:rows, :])
                        if l < n_layers - 1:
                            for p, after_g in piece_group:
                                if after_g == g:
                                    lo, hi = int(bounds[p]), int(bounds[p + 1])
                                    nc.gpsimd.collective_compute(
                                        "AllGather", OP.bypass,
                                        replica_groups=rgroups,
                                        ins=[ag_in[l][lo:hi, :]],
                                        outs=[hbuf[l + 1][ncores * lo:
                                                          ncores * hi, :]])

    nc.compile()
    return nc


# ---------------------------------------------------------------- runner
def run(x, edge_index, conv_w, conv_b, bn_gamma, bn_beta, bn_mean, bn_var,
        lin_w, lin_b, *, trace=False):
    n_nodes, d = x.shape
    n_layers = conv_w.shape[0]
    n_cls = lin_w.shape[1]
    assert d == P and n_nodes % NCORES == 0
    sh = n_nodes // NCORES
    t_tiles = math.ceil(sh / P)

    per_core, layout = preprocess_edges(edge_index, n_nodes)
    nc = build_program(n_nodes, n_layers, n_cls, layout)

    from ml_dtypes import bfloat16
    dinv = layout["dinv"]
    xt = np.asarray(x, dtype=np.float32) * dinv[:, None]   # T = dinv_s * x
    xbf = np.ascontiguousarray(xt.astype(bfloat16))
    xbf_r = np.empty_like(xbf)
    xbf_r[layout["posarr"]] = xbf
    shared = {
        "xbf": xbf_r,
        "conv_w": np.ascontiguousarray(np.asarray(conv_w, dtype=np.float32)),
        "conv_b": np.ascontiguousarray(np.asarray(conv_b, dtype=np.float32)),
        "bn_gamma": np.ascontiguousarray(np.asarray(bn_gamma, dtype=np.float32)),
        "bn_beta": np.ascontiguousarray(np.asarray(bn_beta, dtype=np.float32)),
        "bn_mean": np.ascontiguousarray(np.asarray(bn_mean, dtype=np.float32)),
        "bn_var": np.ascontiguousarray(np.asarray(bn_var, dtype=np.float32)),
        "lin_w": np.ascontiguousarray(np.asarray(lin_w, dtype=np.float32)),
        "lin_b_rep": np.ascontiguousarray(
            np.broadcast_to(np.asarray(lin_b, dtype=np.float32), (P, n_cls))),
    }
    in_maps = []
    for c in range(NCORES):
        dl = np.zeros(t_tiles * P, np.float32)
        dl[:sh] = dinv[c * sh:(c + 1) * sh]
        dinvT = np.ascontiguousarray(
            np.broadcast_to(dl.astype(bfloat16), (P, t_tiles * P)))
        edinv = np.ascontiguousarray(dl.reshape(t_tiles, P).T)
        in_maps.append(dict(
            shared,
            xself=np.ascontiguousarray(xbf[c * sh:(c + 1) * sh]),
            dinvT=dinvT, edinv=edinv,
            **per_core[c]))
    res = run_bass_kernel_spmd(nc, in_maps, list(range(NCORES)), trace=trace)
    out = np.concatenate([np.asarray(res.results[c]["out"])
                          for c in range(NCORES)], axis=0)
    return out, res


def kernel(x, edge_index, conv_w, conv_b, bn_gamma, bn_beta, bn_mean, bn_var,
           lin_w, lin_b):
    out, _ = run(x, edge_index, conv_w, conv_b, bn_gamma, bn_beta,
                 bn_mean, bn_var, lin_w, lin_b,
                 trace=bool(int(os.environ.get("JKNET_TRACE", "0"))))
    return out
